# revision 15
# baseline (speedup 1.0000x reference)
"""Trainium2 Bass kernel for a 2-layer GRU teacher-forced decoder.

Device strategy (8 cores, SPMD, no collectives):
  - The sequential GRU recurrence is replicated on every core; the output
    projection [T*B, H] @ [H, V] is sharded column-wise (vocab) 8 ways.
  - All matmuls and gate element-wise math run in bf16 (gate pre-activations
    accumulate in fp32 PSUM); host-emulated rel err ~6e-3 (threshold 2e-2).

Structure (v2, driven by NTFF profiles of earlier versions):
  - Layer 0 and layer 1 run in ONE software-pipelined loop (L1 one step
    behind L0).  The PE engine queue is strict FIFO, so per iteration the
    MMs are emitted in readiness order: L1-input(t-1), L1-rec(t-1),
    L0-rec(t), L0-input-copies(t+1), then the two transpose groups.  Each
    layer's ~5us of independent MMs covers the other layer's element-wise
    chain; PE-idle gaps stay under the ~3.4us HAM window (the
    phase-sequential version lost ~0.5ms to HAM re-throttling at half
    clock).
  - Quarter-packed gates: [128, 1024] PSUM = [(q, b), (gate, j)], quarters
    on PE column groups via tile_position=(0, 32q); MM loops are k-OUTER /
    quarter-INNER so the four column-group streams run concurrently
    (quarter-outer order serializes them -> 4x PE time).
  - gi0 = relu(emb[tok]) @ W_ih0.T (+ all layer-0 rz/C biases) is
    precomputed on the HOST, streamed bf16 from HBM, and injected into the
    PSUM accumulation with K=32 identity copy-matmuls (stationary at base
    partition 0 -- row-offset stationaries crash the exec unit on this
    stack).  Removes the L0 input GEMM from the PE and frees SBUF so both
    layers' weights stay resident.
  - Remaining gate biases enter PSUM via K=1 matmuls (ones[1,32] stationary,
    bias row moving), so sigmoid/tanh inputs are complete in PSUM and the
    sigmoids read PSUM directly on the Scalar engine.
  - h' = n + z*(h - n); sigmoid split into r and z ops so the tanh chain
    starts one ACT-op earlier; PSUM->SBUF evacuations (HT casts) run on the
    Scalar engine AFTER both layers' activations to keep ACT/DVE FIFOs
    unblocked.
"""

import os
import sys
import numpy as np

sys.path.insert(0, "/opt/trn_rl_repo")

import ml_dtypes

V, E, H, B, ML = 32000, 512, 1024, 32, 64
SOS = 1
T = ML - 1          # 63
TB = T * B          # 2016
NCORES = 8
VS = V // NCORES    # 4000 vocab slice per core
Q = 4               # hidden quarters
J = H // Q          # 256
KH = H // 128       # 8 contraction chunks over H
MT = 126            # logits M-tile (2016 = 16 * 126)
GI_RING = 3         # gi0 prefetch ring depth
H0_RING = 4         # h0T history ring depth

_BF = ml_dtypes.bfloat16


def _bf16(x):
    return np.asarray(x, np.float32).astype(_BF)


def _prep_wT(w, kchunks):
    wt = np.ascontiguousarray(np.asarray(w, np.float32).T)       # [K, 3H]
    wt = wt.reshape(kchunks, 128, wt.shape[1]).transpose(1, 0, 2)  # [128, k, 3H]
    return _bf16(wt)


def _prep_hq(h):
    hq = np.asarray(h, np.float32).reshape(B, Q, J).transpose(1, 0, 2).reshape(Q * B, J)
    return _bf16(np.ascontiguousarray(hq))


def _prep_hT(h):
    ht = np.asarray(h, np.float32).T.reshape(KH, 128, B).transpose(1, 0, 2)
    return _bf16(ht)


def _bias_rows(b_ih, b_hh):
    """brz [1,Q,2J] = (bi+bh) rz per quarter; bhn/bcr [1,Q,J] = bh_n / bi_n."""
    bi = np.asarray(b_ih, np.float32)
    bh = np.asarray(b_hh, np.float32)
    comb = bi + bh
    brz = np.empty((1, Q, 2 * J), np.float32)
    bhn = np.empty((1, Q, J), np.float32)
    bcr = np.empty((1, Q, J), np.float32)
    for q in range(Q):
        s = q * J
        brz[0, q, 0:J] = comb[s:s + J]
        brz[0, q, J:2 * J] = comb[H + s:H + s + J]
        bhn[0, q] = bh[2 * H + s:2 * H + s + J]
        bcr[0, q] = bi[2 * H + s:2 * H + s + J]
    return _bf16(brz), _bf16(bhn), _bf16(bcr)


def _build_inputs(encoder_hidden, target_tensor, emb,
                  W_ih0, W_hh0, b_ih0, b_hh0, W_ih1, W_hh1, b_ih1, b_hh1,
                  W_out, b_out):
    tt = np.asarray(target_tensor)
    toks = np.concatenate(
        [np.full((B, 1), SOS, dtype=tt.dtype), tt[:, 1:ML - 1]], axis=1).T  # [T, B]
    X = np.maximum(np.asarray(emb, np.float32)[toks], 0.0)       # [T, B, E]
    # gi0 exactly as the device would compute it: bf16 x, bf16 W, fp32 accum
    Xb = _bf16(X).astype(np.float32).reshape(TB, E)
    Wb = _bf16(np.asarray(W_ih0, np.float32)).astype(np.float32)
    gi0 = Xb @ Wb.T                                               # [TB, 3H] fp32
    bi = np.asarray(b_ih0, np.float32)
    bh = np.asarray(b_hh0, np.float32)
    gi0[:, 0:2 * H] += (bi + bh)[None, 0:2 * H]
    gi0[:, 2 * H:] += bi[None, 2 * H:]
    gi0 = gi0.reshape(T, B, 3 * H)
    # quarter layout [128=(q,b), T, 768] with cols [r_q | z_q | n_q]
    g = np.empty((Q, B, T, 3 * J), np.float32)
    for q in range(Q):
        s = q * J
        g[q, :, :, 0:J] = gi0[:, :, s:s + J].transpose(1, 0, 2)
        g[q, :, :, J:2 * J] = gi0[:, :, H + s:H + s + J].transpose(1, 0, 2)
        g[q, :, :, 2 * J:] = gi0[:, :, 2 * H + s:2 * H + s + J].transpose(1, 0, 2)
    gi0q = np.ascontiguousarray(_bf16(g.reshape(128, T, 3 * J)))

    _, bhn0, _ = _bias_rows(b_ih0, b_hh0)
    brz1, bhn1, bcr1 = _bias_rows(b_ih1, b_hh1)

    ident = np.zeros((128, 32), np.float32)
    for gg in range(4):
        ident[gg * 32:(gg + 1) * 32] = np.eye(32, dtype=np.float32)

    shared = {
        "gi0": gi0q,
        "h0q": _prep_hq(encoder_hidden[0]),
        "h1q": _prep_hq(encoder_hidden[1]),
        "h0T": _prep_hT(encoder_hidden[0]),
        "h1T": _prep_hT(encoder_hidden[1]),
        "whh0T": _prep_wT(W_hh0, KH),
        "wih1T": _prep_wT(W_ih1, KH),
        "whh1T": _prep_wT(W_hh1, KH),
        "bhn0": bhn0,
        "brz1": brz1, "bhn1": bhn1, "bcr1": bcr1,
        "ident": _bf16(ident),
        "ones": _bf16(np.ones((1, 32), np.float32)),
    }
    wout = np.asarray(W_out, np.float32)
    per_core = []
    for c in range(NCORES):
        sl = slice(c * VS, (c + 1) * VS)
        woutT = wout[sl].T.reshape(KH, 128, VS).transpose(1, 0, 2)
        per_core.append({"woutT": np.ascontiguousarray(_bf16(woutT))})
    return shared, per_core


# ---------------------------------------------------------------------------
# Device program
# ---------------------------------------------------------------------------

def _emit(nc, tc, io, n_steps=T):
    from concourse import mybir
    from concourse.alu_op_type import AluOpType as alu

    f32 = mybir.dt.float32
    bf16 = mybir.dt.bfloat16
    Sig = mybir.ActivationFunctionType.Sigmoid
    Tanh = mybir.ActivationFunctionType.Tanh

    ctx_pools = []
    pool_cms = {}

    def pool(name, bufs, space="SBUF"):
        p = tc.tile_pool(name=name, bufs=bufs, space=space)
        ctx_pools.append(p)
        pool_cms[name] = p
        return p.__enter__()

    consts = pool("consts", 1)
    state = pool("state", 1)
    w0p = pool("w0p", 1)
    arena_p = pool("arena", 1)
    hqp = pool("hq", 2)
    work = pool("work", 2)
    gip = pool("gip", GI_RING)
    stp = pool("stp", 1)

    ident_sb = consts.tile([128, 32], bf16)
    nc.sync.dma_start(ident_sb[:], io["ident"][:])
    ones_sb = consts.tile([1, 32], bf16, tag="ones")
    nc.sync.dma_start(ones_sb[:], io["ones"][:])
    bias_rows = {}
    for nm in ("bhn0", "brz1", "bhn1", "bcr1"):
        width = 2 * J if nm.startswith("brz") else J
        tl = consts.tile([1, Q, width], bf16, tag=nm, name=nm)
        nc.sync.dma_start(tl[:], io[nm][:])
        bias_rows[nm] = tl

    # h0: small ring (only the last few steps are live); h1: full history
    # (consumed by the trailing logits GEMM).
    h0T = state.tile([128, KH, H0_RING * 32], bf16, tag="h0T", name="h0T")
    nc.sync.dma_start(h0T[:, :, 0:32], io["h0T"][:])
    h1T = state.tile([128, KH, (n_steps + 1) * 32], bf16, tag="h1T", name="h1T")
    nc.sync.dma_start(h1T[:, :, 0:32], io["h1T"][:])

    hq_init = {}
    for L in (0, 1):
        hq_init[L] = consts.tile([128, J], bf16, tag=f"hq{L}i", name=f"hq{L}i")
        nc.sync.dma_start(hq_init[L][:], io[f"h{L}q"][:])

    # ---------------- per-step emission helpers ----------------

    def l1_input(psG1, t):
        """L1 input-path MMs (lhsT = h0 at step t+1) + C bias."""
        G = psG1.tile([128, 4 * J], f32, tag="G1", name="G1")
        slot = (t + 1) % H0_RING
        for k in range(KH):
            lhsT = h0T[:, k, slot * 32:slot * 32 + 32]
            w3 = a1[:, k, :].rearrange("p (g j) -> p g j", g=3)
            for q in range(Q):
                tp = (0, 32 * q)
                nc.tensor.matmul(
                    G[32 * q:32 * q + 32, 0:2 * J].rearrange("p (g j) -> p g j", g=2),
                    lhsT, w3[:, 0:2, q * J:(q + 1) * J],
                    start=(k == 0), stop=False, tile_position=tp,
                    skip_group_check=True)
                nc.tensor.matmul(G[32 * q:32 * q + 32, 3 * J:4 * J],
                                 lhsT, w3[:, 2, q * J:(q + 1) * J],
                                 start=(k == 0), stop=False,
                                 tile_position=tp, skip_group_check=True)
        for q in range(Q):
            nc.tensor.matmul(G[32 * q:32 * q + 32, 3 * J:4 * J],
                             ones_sb[:], bias_rows["bcr1"][:, q, :],
                             start=False, stop=True, tile_position=(0, 32 * q),
                             skip_group_check=True)
        return G

    def rec_mms(G, Wa, wofs, layer, hT, t_or_slot):
        """Recurrent-path MMs.  Layer 0 has no PSUM input path, so its k==0
        rz MM opens the group (start=True) and k==KH-1 closes it (rz bias is
        host-folded into gi0); layer 1 closes rz with the brz1 K=1 row.
        Both layers close hn with their bhn row."""
        for k in range(KH):
            lhsT = hT[:, k, t_or_slot * 32:t_or_slot * 32 + 32]
            w3 = Wa[:, wofs + k, :].rearrange("p (g j) -> p g j", g=3)
            for q in range(Q):
                tp = (0, 32 * q)
                nc.tensor.matmul(
                    G[32 * q:32 * q + 32, 0:2 * J].rearrange("p (g j) -> p g j", g=2),
                    lhsT, w3[:, 0:2, q * J:(q + 1) * J],
                    start=(layer == 0 and k == 0),
                    stop=(layer == 0 and k == KH - 1),
                    tile_position=tp, skip_group_check=True)
                nc.tensor.matmul(G[32 * q:32 * q + 32, 2 * J:3 * J],
                                 lhsT, w3[:, 2, q * J:(q + 1) * J],
                                 start=(k == 0), stop=False,
                                 tile_position=tp, skip_group_check=True)
        for q in range(Q):
            if layer == 1:
                nc.tensor.matmul(G[32 * q:32 * q + 32, 0:2 * J],
                                 ones_sb[:], bias_rows["brz1"][:, q, :],
                                 start=False, stop=True,
                                 tile_position=(0, 32 * q),
                                 skip_group_check=True)
            nc.tensor.matmul(G[32 * q:32 * q + 32, 2 * J:3 * J],
                             ones_sb[:], bias_rows[f"bhn{layer}"][:, q, :],
                             start=False, stop=True, tile_position=(0, 32 * q),
                             skip_group_check=True)

    def elem_chain(G, hq_prev, layer, gi_t=None):
        """r/z sigmoids, tanh, h' = n + z*(h-n) (DVE bf16).

        Layer 1 reads complete pre-activations straight from PSUM; layer 0
        first adds the host-precomputed gi0 slice (rz), and takes the n-gate
        input part from SBUF."""
        if layer == 0:
            sp = work.tile([128, 2 * J], bf16, tag="sp")
            nc.vector.tensor_tensor(sp[:], G[:, 0:2 * J], gi_t[:, 0:2 * J],
                                    alu.add)
            rz_src = sp
        else:
            rz_src = G
        r_t = work.tile([128, J], bf16, tag="r")
        nc.scalar.activation(r_t[:], rz_src[:, 0:J], Sig)
        z_t = work.tile([128, J], bf16, tag="z")
        nc.scalar.activation(z_t[:], rz_src[:, J:2 * J], Sig)
        t1 = work.tile([128, J], bf16, tag="tmp", bufs=4)
        nc.vector.tensor_tensor(t1[:], r_t[:], G[:, 2 * J:3 * J], alu.mult)
        t3 = work.tile([128, J], bf16, tag="tmp", bufs=4)
        c_src = gi_t[:, 2 * J:3 * J] if layer == 0 else G[:, 3 * J:4 * J]
        nc.vector.tensor_tensor(t3[:], t1[:], c_src, alu.add)
        n_t = work.tile([128, J], bf16, tag="n")
        nc.scalar.activation(n_t[:], t3[:], Tanh)
        d_t = work.tile([128, J], bf16, tag="tmp", bufs=4)
        nc.vector.tensor_tensor(d_t[:], hq_prev[:], n_t[:], alu.subtract)
        u_t = work.tile([128, J], bf16, tag="tmp", bufs=4)
        nc.vector.tensor_tensor(u_t[:], z_t[:], d_t[:], alu.mult)
        hq_new = hqp.tile([128, J], bf16, tag=f"hq{layer}")
        nc.vector.tensor_tensor(hq_new[:], n_t[:], u_t[:], alu.add)
        # stage quarters to base partition 0 (PE stationary restriction)
        st = stp.tile([32, H], bf16, tag="st", name="st")
        for q in range(Q):
            nc.sync.dma_start(st[0:32, q * J:(q + 1) * J],
                              hq_new[32 * q:32 * q + 32, :])
        return hq_new, st

    def transposes(psT, st):
        TP = psT.tile([128, KH * 32], bf16, tag="TP")
        for k in range(KH):
            nc.tensor.matmul(TP[:, 32 * k:32 * k + 32],
                             st[0:32, 128 * k:128 * (k + 1)],
                             ident_sb[0:32, :], is_transpose=True,
                             skip_group_check=True)
        return TP

    def cast_out(TP, layer, t):
        if layer == 0:
            dst = h0T[:, :, ((t + 1) % H0_RING) * 32:((t + 1) % H0_RING) * 32 + 32]
        else:
            dst = h1T[:, :, (t + 1) * 32:(t + 2) * 32]
        nc.scalar.copy(dst, TP[:].rearrange("p (k b) -> p k b", k=KH))

    # ---------------- main pipelined loop ----------------
    psG0 = pool("psG0", 2, space="PSUM")
    psG1 = pool("psG1", 1, space="PSUM")
    psT = pool("psT", 1, space="PSUM")

    w0 = w0p.tile([128, KH, 3 * H], bf16, tag="w0", name="w0")
    nc.sync.dma_start(w0[:], io["whh0T"][:])
    a1 = arena_p.tile([128, 2 * KH, 3 * H], bf16, tag="arena", name="a1")
    nc.sync.dma_start(a1[:, 0:KH, :], io["wih1T"][:])
    nc.sync.dma_start(a1[:, KH:2 * KH, :], io["whh1T"][:])

    gis = []

    def load_gi(t):
        g = gip.tile([128, 3 * J], bf16, tag="gi")
        nc.sync.dma_start(g[:], io["gi0"][:, t, :])
        gis.append(g)

    for t in range(min(2, n_steps)):
        load_gi(t)

    hq0 = hq_init[0]
    hq1 = hq_init[1]
    G1 = None

    for t in range(n_steps):
        # --- PE stream, readiness order ---
        if t >= 1:
            G1 = l1_input(psG1, t - 1)
            rec_mms(G1, a1, KH, 1, h1T, t - 1)
        G0 = psG0.tile([128, 4 * J], f32, tag="G0", name="G0")
        rec_mms(G0, w0, 0, 0, h0T, t % H0_RING)
        if t + 2 < n_steps:
            load_gi(t + 2)
        # --- element-wise chains (L1 step t-1 first: its MMs finish first) ---
        if t >= 1:
            hq1, st1 = elem_chain(G1, hq1, 1)
        hq0, st0 = elem_chain(G0, hq0, 0, gi_t=gis[t])
        if t >= 1:
            TP1 = transposes(psT, st1)
            cast_out(TP1, 1, t - 1)
        TP0 = transposes(psT, st0)
        cast_out(TP0, 0, t)

    # L1 epilogue step (t = n_steps-1)
    if n_steps >= 1:
        G1 = l1_input(psG1, n_steps - 1)
        rec_mms(G1, a1, KH, 1, h1T, n_steps - 1)
        hq1, st1 = elem_chain(G1, hq1, 1)
        TP1 = transposes(psT, st1)
        cast_out(TP1, 1, n_steps - 1)

    for nm in ("psT", "psG1", "psG0", "stp", "gip", "work", "hq"):
        cm = pool_cms[nm]
        ctx_pools.remove(cm)
        cm.__exit__(None, None, None)

    # ================= logits GEMM (vocab-sharded) =================
    NS = 500
    with tc.tile_pool(name="psum2", bufs=2, space="PSUM") as psum2, \
         tc.tile_pool(name="outp", bufs=2) as outp:
        a2 = arena_p.tile([128, KH, VS], bf16, tag="arena", name="a2")
        nc.sync.dma_start(a2[:], io["woutT"][:])
        n_mt = (n_steps * B + MT - 1) // MT
        for m in range(n_mt):
            rows = min(MT, n_steps * B - m * MT)
            for s in range(VS // NS):
                Lg = psum2.tile([128, NS], f32, tag="L", name="L")
                for k in range(KH):
                    nc.tensor.matmul(
                        Lg[0:rows, :],
                        h1T[:, k, 32 + m * MT:32 + m * MT + rows],
                        a2[:, k, s * NS:(s + 1) * NS],
                        start=(k == 0), stop=(k == KH - 1))
                ob = outp.tile([128, NS], f32, tag="ob", name="ob")
                nc.vector.tensor_copy(ob[0:rows, :], Lg[0:rows, :])
                nc.sync.dma_start(
                    io["logits"][m * MT:m * MT + rows, s * NS:(s + 1) * NS],
                    ob[0:rows, :])

    for p in reversed(ctx_pools):
        p.__exit__(None, None, None)


def _build_program(n_steps=T):
    import concourse.bacc as bacc
    import concourse.tile as tile
    from concourse import mybir

    f32 = mybir.dt.float32
    bf16 = mybir.dt.bfloat16

    nc = bacc.Bacc("TRN2", target_bir_lowering=False, debug=False,
                   num_devices=NCORES)

    def din(name, shape, dt):
        return nc.dram_tensor(name, list(shape), dt, kind="ExternalInput").ap()

    io = {
        "gi0": din("gi0", (128, T, 3 * J), bf16),
        "h0q": din("h0q", (128, J), bf16),
        "h1q": din("h1q", (128, J), bf16),
        "h0T": din("h0T", (128, KH, 32), bf16),
        "h1T": din("h1T", (128, KH, 32), bf16),
        "whh0T": din("whh0T", (128, KH, 3 * H), bf16),
        "wih1T": din("wih1T", (128, KH, 3 * H), bf16),
        "whh1T": din("whh1T", (128, KH, 3 * H), bf16),
        "bhn0": din("bhn0", (1, Q, J), bf16),
        "brz1": din("brz1", (1, Q, 2 * J), bf16),
        "bhn1": din("bhn1", (1, Q, J), bf16),
        "bcr1": din("bcr1", (1, Q, J), bf16),
        "ident": din("ident", (128, 32), bf16),
        "ones": din("ones", (1, 32), bf16),
        "woutT": din("woutT", (128, KH, VS), bf16),
        "logits": nc.dram_tensor("logits", [TB, VS], f32,
                                 kind="ExternalOutput").ap(),
    }

    with tile.TileContext(nc) as tc:
        _emit(nc, tc, io, n_steps=n_steps)

    nc.compile()
    return nc


_CACHED = {}


def _get_program(n_steps=T):
    if n_steps not in _CACHED:
        _CACHED[n_steps] = _build_program(n_steps)
    return _CACHED[n_steps]


def kernel(encoder_outputs, encoder_hidden, target_tensor, emb,
           W_ih0, W_hh0, b_ih0, b_hh0, W_ih1, W_hh1, b_ih1, b_hh1,
           W_out, b_out, _trace=False):
    from concourse import bass_utils

    shared, per_core = _build_inputs(
        encoder_hidden, target_tensor, emb,
        W_ih0, W_hh0, b_ih0, b_hh0, W_ih1, W_hh1, b_ih1, b_hh1, W_out, b_out)

    nc = _get_program()
    in_maps = []
    for c in range(NCORES):
        m = dict(shared)
        m.update(per_core[c])
        in_maps.append(m)

    res = None
    for attempt in range(3):
        try:
            res = bass_utils.run_bass_kernel_spmd(
                nc, in_maps, core_ids=list(range(NCORES)), trace=_trace)
            break
        except Exception:
            if attempt == 2:
                raise
            import time
            time.sleep(20)

    parts = [res.results[c]["logits"].reshape(T, B, VS) for c in range(NCORES)]
    full = np.concatenate(parts, axis=2)          # [T, B, V]
    full += np.asarray(b_out, np.float32)[None, None, :]
    out = np.ascontiguousarray(full.transpose(1, 0, 2)).astype(np.float32)
    if _trace:
        kernel.last_results = res
    return out


kernel.last_results = None
